# revision 67
# baseline (speedup 1.0000x reference)
"""Trainium2 Bass kernel for nn_CGLayer (PointNet++-style set-abstraction layer).

Pipeline per NeuronCore (data-parallel: core c -> batch c//2, half c%2 of M):
  1. shift MLP (replicated, BN stats are permutation-invariant)
  2. ball query: d2 via PE matmul (5-dim augmented contraction) -> fused
     DVE pass u = (d2<1) * (N - n); first-32 extraction with max8/match_replace
     on a depth schedule over 512-wide segments; merge; decode.
  3. Hfull[n,:] = W1f . feat_n + W1x . xyz_n  (fp16, staged in DRAM),
     dma_gather(transpose=True) lands [channel, point] tiles directly.
  4. 3-layer MLP with fp16 activations resident in one SBUF buffer (in-place
     across layers), training-mode BN via per-core sums + tiny AllReduce,
     BN+ReLU fused into single ACT pass; max-pool over K; PE-transpose out.
"""
import numpy as np

import concourse.bass as bass
import concourse.mybir as mybir
from concourse.tile import TileContext
from concourse.tile_rust import add_dep_helper
from concourse.masks import make_identity
from concourse import library_config

f32 = mybir.dt.float32
f16 = mybir.dt.float16
i16 = mybir.dt.int16
AL = mybir.AluOpType
AF = mybir.ActivationFunctionType
AX = mybir.AxisListType

B, N, M, C, K = 4, 16384, 1024, 256, 32
NCORES = 8
O = 512
EPS = 1e-5


def _depths(nseg):
    return [32 if j < 4 else (16 if j < 12 else 8) for j in range(nseg)]


_LIB_DEPS = {}


def build(n=N, qpc=M * B // NCORES, ncores=NCORES, bm=B * M, use_cc=True, use_gather=True,
          debug=False):
    nseg = n // 512
    depths = _depths(nseg)
    ncand = sum(depths)
    nqt = qpc // 128
    xt = qpc * K                  # points per core
    ng = xt // 1024               # gather calls
    cnt = float(ncores * xt)      # global BN count
    nfc = bm // 512               # shift-layer free chunks

    nc = bass.Bass()
    faug = nc.dram_tensor("faug", [C + 3, n], f16, kind="ExternalInput")
    yaug = nc.dram_tensor("yaug", [15, n], f16, kind="ExternalInput")
    fshh = nc.dram_tensor("fshh", [C, bm], f16, kind="ExternalInput")
    fshl = nc.dram_tensor("fshl", [C, bm], f16, kind="ExternalInput")
    xyzt = nc.dram_tensor("xyzt", [3, bm], f32, kind="ExternalInput")
    w1aug = nc.dram_tensor("w1aug", [C + 3, O], f16, kind="ExternalInput")
    w2t_d = nc.dram_tensor("w2t", [O, O], f16, kind="ExternalInput")
    w3t_d = nc.dram_tensor("w3t", [O, O], f16, kind="ExternalInput")
    sw1t2_d = nc.dram_tensor("sw1t2", [C, 256], f16, kind="ExternalInput")
    sw2t_d = nc.dram_tensor("sw2t", [128, 3], f32, kind="ExternalInput")
    bnp_d = nc.dram_tensor("bnp", [128, 28], f32, kind="ExternalInput")
    out_d = nc.dram_tensor("out", [128, 4, qpc], f16, kind="ExternalOutput")
    if debug:
        dbg_new3 = nc.dram_tensor("dbg_new3", [3, qpc], f32, kind="ExternalOutput")
        dbg_gidx = nc.dram_tensor("dbg_gidx", [128, qpc * 32 // 16], i16,
                                  kind="ExternalOutput")
        dbg_xaug = nc.dram_tensor("dbg_xaug", [15, qpc], f16, kind="ExternalOutput")
    hfull = nc.dram_tensor("hfull", [n, O], f16)
    stat_io = [
        (nc.dram_tensor(f"stat_in{l}", [128, 8], f32),
         nc.dram_tensor(f"stat_out{l}", [128, 8], f32, addr_space="Shared"))
        for l in range(3)
    ]

    with TileContext(nc) as tc:
        with tc.tile_pool(name="persist", bufs=1) as pp:
            ident32 = pp.tile([128, 128], f32)
            make_identity(nc, ident32)
            ident16 = pp.tile([128, 128], f16)
            make_identity(nc, ident16)

            w2t = pp.tile([128, 4, O], f16)
            nc.sync.dma_start(out=w2t, in_=w2t_d.rearrange("(c p) o -> p c o", p=128))
            w3t = pp.tile([128, 4, O], f16)
            nc.sync.dma_start(out=w3t, in_=w3t_d.rearrange("(c p) o -> p c o", p=128))
            w1a0 = pp.tile([128, O], f16)
            nc.sync.dma_start(out=w1a0, in_=w1aug[0:128, :])
            w1a1 = pp.tile([128, O], f16)
            nc.sync.dma_start(out=w1a1, in_=w1aug[128:256, :])
            w1a2 = pp.tile([35, O], f16)
            nc.sync.dma_start(out=w1a2[0:3, :], in_=w1aug[256:259, :])
            nc.sync.dma_start(out=w1a2[32:35, :], in_=w1aug[256:259, :])
            bnp = pp.tile([128, 28], f32)
            nc.sync.dma_start(out=bnp, in_=bnp_d[:, :])

            gidx = pp.tile([128, xt // 16], i16)
            nc.vector.memset(gidx, 0)
            qs = pp.tile([128, 4, qpc], f16)
            pooled = pp.tile([128, 4, qpc], f16)
            praw = pp.tile([128, 4, qpc], f16)
            s1acc = pp.tile([128, 4 * 8 * ng], f32)
            s2acc = pp.tile([128, 4 * ng], f32)
            scl = [pp.tile([128, 4], f32, name=f'scl{i}') for i in range(3)]
            bia = [pp.tile([128, 4], f32, name=f'bia{i}') for i in range(3)]
            stpk = pp.tile([128, 8], f32)
            eps128 = pp.tile([128, 1], f32)
            nc.vector.memset(eps128, EPS)
            stg = pp.tile([128, 8], f32)

            # ---------------- phase 1: shift layer + ball query + Hfull ------
            with tc.tile_pool(name="bq", bufs=1) as bq, \
                 tc.tile_pool(name="bqs", bufs=2) as bqs, \
                 tc.tile_pool(name="ps1", bufs=2, space="PSUM") as ps1, \
                 tc.tile_pool(name="pshf", bufs=2, space="PSUM") as pshf, \
                 tc.tile_pool(name="psd2", bufs=2, space="PSUM") as psd2:
                # --- shift layer (replicated over all queries) ---

                sw1t_sb = bq.tile([128, 2, 256], f16)
                nc.sync.dma_start(out=sw1t_sb, in_=sw1t2_d.rearrange("(c p) o -> p c o", p=128))
                sw2t_sb = bq.tile([128, 3], f32)
                nc.sync.dma_start(out=sw2t_sb, in_=sw2t_d[:, :])
                xyzt_sb = bq.tile([3, qpc], f32)
                nc.sync.dma_start(out=xyzt_sb, in_=xyzt[:, 0:qpc])

                # h1 via exact fp16 hi/lo split: swh.fh + swh.fl + swl.fh
                h1 = bq.tile([128, bm], f32)
                fshhr = fshh.rearrange("(c p) m -> p c m", p=128)
                fshlr = fshl.rearrange("(c p) m -> p c m", p=128)
                for fc in range(nfc):
                    ph = ps1.tile([128, 512], f32, tag="mx")
                    fh = bqs.tile([128, 2, 512], f16, tag="fshh")
                    nc.sync.dma_start(out=fh, in_=fshhr[:, :, fc * 512:(fc + 1) * 512])
                    fl = bqs.tile([128, 2, 512], f16, tag="fshl")
                    nc.sync.dma_start(out=fl, in_=fshlr[:, :, fc * 512:(fc + 1) * 512])
                    for kc in range(2):
                        nc.tensor.matmul(ph, sw1t_sb[:, kc, 0:128], fh[:, kc],
                                         start=(kc == 0), stop=False)
                        nc.tensor.matmul(ph, sw1t_sb[:, kc, 0:128], fl[:, kc],
                                         start=False, stop=False)
                        nc.tensor.matmul(ph, sw1t_sb[:, kc, 128:256], fh[:, kc],
                                         start=False, stop=(kc == 1))
                    nc.scalar.activation(h1[:, fc * 512:(fc + 1) * 512], ph, AF.Copy)
                bst1 = bq.tile([128, nfc, 6], f32)
                for fc in range(nfc):
                    nc.vector.bn_stats(bst1[:, fc], h1[:, fc * 512:(fc + 1) * 512])
                bag1 = bq.tile([128, 2], f32)
                nc.vector.bn_aggr(bag1, bst1)
                std1 = bq.tile([128, 1], f32)
                nc.scalar.activation(std1, bag1[:, 1:2], AF.Sqrt, bias=eps128[:, 0:1])
                rstd1 = bq.tile([128, 1], f32)
                nc.vector.reciprocal(rstd1, std1)
                sc_sh = bq.tile([128, 1], f32)
                nc.vector.tensor_mul(sc_sh, rstd1, bnp[:, 0:1])
                tmp1 = bq.tile([128, 1], f32)
                nc.vector.tensor_mul(tmp1, bag1[:, 0:1], sc_sh)
                bi_sh = bq.tile([128, 1], f32)
                nc.vector.tensor_sub(bi_sh, bnp[:, 1:2], tmp1)
                a_sh = bq.tile([128, bm], f32)
                nc.scalar.activation(a_sh, h1, AF.Relu, bias=bi_sh, scale=sc_sh)

                h2 = bq.tile([3, bm], f32)
                for fc in range(nfc):
                    ph2 = ps1.tile([3, 512], f32, tag="mx")
                    nc.tensor.matmul(ph2, sw2t_sb, a_sh[:, fc * 512:(fc + 1) * 512],
                                     start=True, stop=True)
                    nc.scalar.activation(h2[:, fc * 512:(fc + 1) * 512], ph2, AF.Copy)
                bst2 = bq.tile([3, nfc, 6], f32)
                for fc in range(nfc):
                    nc.vector.bn_stats(bst2[:, fc], h2[:, fc * 512:(fc + 1) * 512])
                bag2 = bq.tile([3, 2], f32)
                nc.vector.bn_aggr(bag2, bst2)
                std2 = bq.tile([3, 1], f32)
                nc.scalar.activation(std2, bag2[:, 1:2], AF.Sqrt, bias=eps128[0:3, 0:1])
                rstd2 = bq.tile([3, 1], f32)
                nc.vector.reciprocal(rstd2, std2)
                sc_s2 = bq.tile([3, 1], f32)
                nc.vector.tensor_mul(sc_s2, rstd2, bnp[0:3, 2:3])
                tmp2 = bq.tile([3, 1], f32)
                nc.vector.tensor_mul(tmp2, bag2[:, 0:1], sc_s2)
                bi_s2 = bq.tile([3, 1], f32)
                nc.vector.tensor_sub(bi_s2, bnp[0:3, 3:4], tmp2)
                new3 = bq.tile([3, qpc], f32)
                nc.scalar.activation(new3, h2[:, 0:qpc], AF.Relu, bias=bi_s2, scale=sc_s2)
                nc.vector.tensor_add(new3, new3, xyzt_sb)

                # --- xaug2: 15-row fp16 hi/lo split of the d2 contraction ---
                # pairs with yaug rows: [yh(3), yl(3), yh(3), ones(3),
                #                        ysq_h, ysq_m, ysq_l]
                # lhsT rows: [-2xh(3), -2xh(3), -2xl(3),
                #             xsqm1_h, xsqm1_m, xsqm1_l, ones(3)]
                xh = bq.tile([3, qpc], f16)
                nc.vector.tensor_copy(xh, new3)
                xl = bq.tile([3, qpc], f16)
                nc.vector.tensor_sub(xl, new3, xh)
                xaug = bq.tile([15, qpc], f16)
                nc.vector.memset(xaug, 1.0)
                nc.vector.tensor_scalar_mul(xaug[0:3, :], xh, -2.0)
                nc.sync.dma_start(out=xaug[3:6, :], in_=xaug[0:3, :])
                t2l = bq.tile([3, qpc], f16)
                nc.vector.tensor_scalar_mul(t2l, xl, -2.0)
                nc.sync.dma_start(out=xaug[6:9, :], in_=t2l)
                sq3 = bq.tile([3, qpc], f32)
                nc.vector.tensor_mul(sq3, new3, new3)
                ones3 = bq.tile([3, 1], f32)
                nc.vector.memset(ones3, 1.0)
                psq = ps1.tile([1, qpc], f32, tag="mx")
                nc.tensor.matmul(psq, ones3, sq3, start=True, stop=True)
                row4 = bq.tile([1, qpc], f32)
                nc.vector.tensor_scalar_add(row4, psq, -1.0)
                xqh = bq.tile([1, qpc], f16)
                nc.vector.tensor_copy(xqh, row4)
                r1 = bq.tile([1, qpc], f32)
                nc.vector.tensor_sub(r1, row4, xqh)
                xqm = bq.tile([1, qpc], f16)
                nc.vector.tensor_copy(xqm, r1)
                r2 = bq.tile([1, qpc], f32)
                nc.vector.tensor_sub(r2, r1, xqm)
                xql = bq.tile([1, qpc], f16)
                nc.vector.tensor_copy(xql, r2)
                nc.sync.dma_start(out=xaug[9:10, :], in_=xqh)
                nc.sync.dma_start(out=xaug[10:11, :], in_=xqm)
                nc.sync.dma_start(out=xaug[11:12, :], in_=xql)

                # --- Q[o, q] = W1x . new_xyz (fp16) ---
                for oc in range(4):
                    pq = ps1.tile([128, qpc], f32, tag="mx")
                    nc.tensor.matmul(pq, w1a2[0:3, oc * 128:(oc + 1) * 128], xh,
                                     start=True, stop=True)
                    nc.scalar.activation(qs[:, oc], pq, AF.Copy)


                # --- ball query ---
                iota_insts = _LIB_DEPS.setdefault('iota', [])
                iota_insts.clear()
                # per-512 sawtooth 512..1 (fp16-exact); merge offsets restore
                # the global priority u = N - n
                iota16f = bq.tile([128, n], f16)
                iota_insts.append(nc.gpsimd.iota(
                    iota16f, pattern=[[0, nseg], [-1, 512]], base=512,
                    channel_multiplier=0, allow_small_or_imprecise_dtypes=True))
                yaug_sb = bq.tile([15, n], f16)
                nc.sync.dma_start(out=yaug_sb, in_=yaug[:, :])
                cand16 = bq.tile([128, ncand], f16)
                cand32 = bq.tile([128, ncand], f32)
                mz = bq.tile([128, ncand], f16)
                offv = bq.tile([128, ncand], f16)
                offj = 0
                for j, dpt in enumerate(depths):
                    nc.vector.memset(offv[:, offj:offj + dpt],
                                     float((nseg - 1 - j) * 512))
                    offj += dpt
                m32 = bq.tile([128, 32], f32)
                idxf = bq.tile([128, 32], f32)
                vm = bq.tile([128, 32], mybir.dt.uint8)
                idx2 = bq.tile([128, 32], f32)
                idxF = bq.tile([128, 32], f32)
                for t in range(nqt):
                    off = 0
                    for ch in range(n // 1024):
                        pd = psd2.tile([128, 1024], f32, tag="pd")
                        for sc in range(2):
                            nc.tensor.matmul(pd[:, sc * 512:(sc + 1) * 512],
                                             xaug[:, t * 128:(t + 1) * 128],
                                             yaug_sb[:, ch * 1024 + sc * 512:ch * 1024 + (sc + 1) * 512],
                                             start=True, stop=True)
                        uch = bqs.tile([128, 1024], f16, tag="uch")
                        nc.vector.scalar_tensor_tensor(
                            uch, pd, 0.0,
                            iota16f[:, ch * 1024:(ch + 1) * 1024],
                            op0=AL.is_lt, op1=AL.mult)
                        for jj in range(2):
                            d = depths[2 * ch + jj]
                            seg = uch[:, jj * 512:(jj + 1) * 512]
                            for r in range(d // 8):
                                nc.vector.max(cand16[:, off:off + 8], seg)
                                if r < d // 8 - 1:
                                    nc.vector.match_replace(seg, cand16[:, off:off + 8], seg, 0.0)
                                off += 8
                    # globalize candidates: valid -> local + (31-j)*512, empty -> -1e9
                    nc.vector.tensor_scalar(mz, cand16, 0.5, None, op0=AL.is_lt)
                    nc.vector.tensor_add(cand32, cand16, offv)
                    nc.vector.scalar_tensor_tensor(cand32, mz, -1e9, cand32,
                                                   op0=AL.mult, op1=AL.add)
                    for r in range(4):
                        nc.vector.max(m32[:, r * 8:(r + 1) * 8], cand32)
                        if r < 3:
                            nc.vector.match_replace(cand32, m32[:, r * 8:(r + 1) * 8], cand32, 0.0)
                    nc.vector.tensor_scalar(idxf, m32, -1.0, float(n),
                                            op0=AL.mult, op1=AL.add)
                    nc.vector.tensor_scalar(vm, idxf, float(n), None, op0=AL.is_lt)
                    nc.vector.select(idx2, vm, idxf, idxf[:, 0:1].to_broadcast([128, 32]))
                    nc.vector.scalar_tensor_tensor(idxF, idx2, float(n), idx2,
                                                   op0=AL.is_lt, op1=AL.mult)
                    pstA = ps1.tile([16, 128], f32, tag="mx")
                    nc.tensor.transpose(pstA, idxF[:, 0:16], ident32)
                    pstB = ps1.tile([16, 128], f32, tag="mx")
                    nc.tensor.transpose(pstB, idxF[:, 16:32], ident32)
                    g2 = gidx.rearrange("p (q two) -> p q two", two=2)
                    nc.vector.tensor_copy(g2[0:16, t * 128:(t + 1) * 128, 0], pstA)
                    nc.vector.tensor_copy(g2[0:16, t * 128:(t + 1) * 128, 1], pstB)
                    for kk in range(1, 8):
                        nc.sync.dma_start(
                            out=gidx[16 * kk:16 * (kk + 1), t * 256:(t + 1) * 256],
                            in_=gidx[0:16, t * 256:(t + 1) * 256])

                if debug:
                    nc.sync.dma_start(out=dbg_new3[:, :], in_=new3)
                    nc.sync.dma_start(out=dbg_xaug[:, :], in_=xaug)
                    nc.sync.dma_start(out=dbg_gidx[:, :], in_=gidx)

                # --- Hfull -> DRAM (fp16) ---
                for g in range(n // 512):
                    fa0 = bqs.tile([128, 512], f16, tag="fa0")
                    nc.sync.dma_start(out=fa0, in_=faug[0:128, g * 512:(g + 1) * 512])
                    fa1 = bqs.tile([128, 512], f16, tag="fa1")
                    nc.sync.dma_start(out=fa1, in_=faug[128:256, g * 512:(g + 1) * 512])
                    fa2 = bqs.tile([3, 512], f16, tag="fa2")
                    nc.sync.dma_start(out=fa2, in_=faug[256:259, g * 512:(g + 1) * 512])
                    hfs4 = bqs.tile([128, 4, O], f16, tag="hfs")
                    for t in range(4):
                        phf = pshf.tile([128, 512], f32, tag="phf")
                        sl = slice(t * 128, (t + 1) * 128)
                        nc.tensor.matmul(phf, fa0[:, sl], w1a0, start=True, stop=False)
                        nc.tensor.matmul(phf, fa1[:, sl], w1a1, start=False, stop=False)
                        nc.tensor.matmul(phf, fa2[:, sl], w1a2[0:3, :], start=False, stop=True)
                        nc.scalar.activation(hfs4[:, t, :], phf, AF.Copy)
                    nc.sync.dma_start(
                        out=hfull[g * 512:(g + 1) * 512, :].rearrange(
                            "(t p) o -> p t o", t=4),
                        in_=hfs4)

            # ---------------- phase 2: gather + MLP ------------------------
            lib_inst = nc.gpsimd.load_library(library_config.mlp)
            for ii in _LIB_DEPS['iota']:
                add_dep_helper(lib_inst.ins, ii.ins, reason="mlp lib after iota")
            with tc.tile_pool(name="mlp", bufs=1) as mp, \
                 tc.tile_pool(name="mps", bufs=2) as mps, \
                 tc.tile_pool(name="psm", bufs=3, space="PSUM") as psm, \
                 tc.tile_pool(name="pso", bufs=2, space="PSUM") as pso:
                b1 = mp.tile([128, 4, xt], f16)

                def stats_to_scale(layer, nslot1, nslot2):
                    nc.vector.tensor_reduce(
                        stpk[:, 0:4].rearrange("p (oc one) -> p oc one", one=1),
                        s1acc[:, 0:4 * nslot1].rearrange("p (oc g) -> p oc g", g=nslot1),
                        axis=AX.X, op=AL.add)
                    nc.vector.tensor_reduce(
                        stpk[:, 4:8].rearrange("p (oc one) -> p oc one", one=1),
                        s2acc[:, 0:4 * nslot2].rearrange("p (oc g) -> p oc g", g=nslot2),
                        axis=AX.X, op=AL.add)
                    finish_stats(layer)

                def finish_stats(layer):
                    wst = nc.sync.dma_start(out=stat_io[layer][0][:, :], in_=stpk)
                    if use_cc:
                        cc = nc.gpsimd.collective_compute(
                            "AllReduce", AL.add,
                            replica_groups=[list(range(ncores))],
                            ins=[stat_io[layer][0][:, :]],
                            outs=[stat_io[layer][1][:, :]])
                        add_dep_helper(cc.ins, wst.ins, reason="cc after stats write")
                        rst = nc.sync.dma_start(out=stg, in_=stat_io[layer][1][:, :])
                        add_dep_helper(rst.ins, cc.ins, reason="stats read after cc")
                    else:
                        rst = nc.sync.dma_start(out=stg, in_=stat_io[layer][0][:, :])
                        add_dep_helper(rst.ins, wst.ins, reason="stats read after write")
                    mean = mp.tile([128, 4], f32, tag=f"mean{layer}")
                    ex2 = mp.tile([128, 4], f32, tag=f"ex2{layer}")
                    nc.vector.tensor_scalar_mul(mean, stg[:, 0:4], 1.0 / cnt)
                    nc.vector.tensor_scalar_mul(ex2, stg[:, 4:8], 1.0 / cnt)
                    msq = mp.tile([128, 4], f32, tag=f"msq{layer}")
                    nc.vector.tensor_mul(msq, mean, mean)
                    var = mp.tile([128, 4], f32, tag=f"var{layer}")
                    nc.vector.tensor_sub(var, ex2, msq)
                    stdt = mp.tile([128, 4], f32, tag=f"std{layer}")
                    nc.scalar.activation(stdt, var, AF.Sqrt, bias=eps128[:, 0:1])
                    rstdt = mp.tile([128, 4], f32, tag=f"rstd{layer}")
                    nc.vector.reciprocal(rstdt, stdt)
                    nc.vector.tensor_mul(scl[layer], rstdt, bnp[:, 4 + 8 * layer:8 + 8 * layer])
                    mb = mp.tile([128, 4], f32, tag=f"mb{layer}")
                    nc.vector.tensor_mul(mb, mean, scl[layer])
                    nc.vector.tensor_sub(bia[layer], bnp[:, 8 + 8 * layer:12 + 8 * layer], mb)

                # --- gather + L1 pre-activations + stats ---
                # non-transpose gather: gt[p, i, :] = Hfull[list[i*128+p], :]
                for g in range(ng):
                    gt = mps.tile([128, 8, O], f16, tag="gt")
                    if use_gather:
                        nc.gpsimd.dma_gather(gt, hfull[:, :], gidx[:, g * 64:(g + 1) * 64],
                                             1024, 1024, O, transpose=False)
                    else:
                        nc.vector.memset(gt, 0.5)
                    for oc in range(4):
                        ptb = pso.tile([128, 1024], f16, tag="ptb")
                        for i in range(8):
                            nc.tensor.transpose(ptb[:, i * 128:(i + 1) * 128],
                                                gt[:, i, oc * 128:(oc + 1) * 128],
                                                ident16)
                        slot = oc * ng + g
                        nc.vector.scalar_tensor_tensor(
                            b1[:, oc, g * 1024:(g + 1) * 1024].rearrange(
                                "p (q k) -> p q k", k=32),
                            ptb.rearrange("p (q k) -> p q k", k=32),
                            0.0,
                            qs[:, oc, g * 32:(g + 1) * 32].rearrange(
                                "p (q one) -> p q one", one=1).to_broadcast([128, 32, 32]),
                            op0=AL.add, op1=AL.subtract,
                            accum_out=s1acc[:, slot:slot + 1])
                    for oc in range(4):
                        sqt = mp.tile([128, 1024], f16, tag="sqt")
                        nc.scalar.activation(
                            sqt, b1[:, oc, g * 1024:(g + 1) * 1024], AF.Square,
                            accum_out=s2acc[:, oc * ng + g:oc * ng + g + 1])
                stats_to_scale(0, ng, ng)

                # --- layer 2 ---
                for g in range(ng):
                    a1 = mps.tile([128, 4, 1024], f16, tag="a1")
                    for oc in range(4):
                        nc.scalar.activation(a1[:, oc], b1[:, oc, g * 1024:(g + 1) * 1024],
                                             AF.Relu, bias=bia[0][:, oc:oc + 1],
                                             scale=scl[0][:, oc:oc + 1])
                    for o2p in range(2):
                        pmA = psm.tile([128, 1024], f32, tag="pm")
                        pmB = psm.tile([128, 1024], f32, tag="pm")
                        o2a, o2b = 2 * o2p, 2 * o2p + 1
                        for oc in range(4):
                            st, sp = (oc == 0), (oc == 3)
                            for xs in range(2):
                                nc.tensor.matmul(pmA[:, xs * 512:(xs + 1) * 512],
                                                 w2t[:, oc, o2a * 128:(o2a + 1) * 128],
                                                 a1[:, oc, xs * 512:(xs + 1) * 512],
                                                 start=st, stop=sp)
                            for xs in range(2):
                                nc.tensor.matmul(pmB[:, xs * 512:(xs + 1) * 512],
                                                 w2t[:, oc, o2b * 128:(o2b + 1) * 128],
                                                 a1[:, oc, xs * 512:(xs + 1) * 512],
                                                 start=st, stop=sp)
                        for o2, pm in ((o2a, pmA), (o2b, pmB)):
                            slot = o2 * ng + g
                            nc.scalar.activation(
                                b1[:, o2, g * 1024:(g + 1) * 1024], pm, AF.Copy,
                                accum_out=s1acc[:, slot:slot + 1])
                    for o2 in range(4):
                        sqt = mp.tile([128, 1024], f16, tag="sqt")
                        nc.vector.scalar_tensor_tensor(
                            sqt, b1[:, o2, g * 1024:(g + 1) * 1024], 1.0,
                            b1[:, o2, g * 1024:(g + 1) * 1024],
                            op0=AL.mult, op1=AL.mult,
                            accum_out=s2acc[:, o2 * ng + g:o2 * ng + g + 1])
                stats_to_scale(1, ng, ng)

                # --- layer 3: stats + maxpool straight from PSUM, BN3 after pool
                # (valid since scale3 = g3/sigma3 > 0 and max commutes with
                #  monotone BN+ReLU)
                for g in range(ng):
                    a1 = mps.tile([128, 4, 1024], f16, tag="a1")
                    for oc in range(4):
                        nc.scalar.activation(a1[:, oc], b1[:, oc, g * 1024:(g + 1) * 1024],
                                             AF.Relu, bias=bia[1][:, oc:oc + 1],
                                             scale=scl[1][:, oc:oc + 1])
                    for o2p in range(2):
                        pmA = psm.tile([128, 1024], f32, tag="pm")
                        pmB = psm.tile([128, 1024], f32, tag="pm")
                        o2a, o2b = 2 * o2p, 2 * o2p + 1
                        for oc in range(4):
                            st, sp = (oc == 0), (oc == 3)
                            for xs in range(2):
                                nc.tensor.matmul(pmA[:, xs * 512:(xs + 1) * 512],
                                                 w3t[:, oc, o2a * 128:(o2a + 1) * 128],
                                                 a1[:, oc, xs * 512:(xs + 1) * 512],
                                                 start=st, stop=sp)
                            for xs in range(2):
                                nc.tensor.matmul(pmB[:, xs * 512:(xs + 1) * 512],
                                                 w3t[:, oc, o2b * 128:(o2b + 1) * 128],
                                                 a1[:, oc, xs * 512:(xs + 1) * 512],
                                                 start=st, stop=sp)
                        for o2, pm in ((o2a, pmA), (o2b, pmB)):
                            slot = o2 * ng + g
                            scr = mps.tile([128, 1024], f16, tag="scr")
                            nc.scalar.activation(scr, pm, AF.Copy,
                                                 accum_out=s1acc[:, slot:slot + 1])
                            sqt = mp.tile([128, 1024], f16, tag="sqt")
                            nc.vector.scalar_tensor_tensor(
                                sqt, scr, 1.0, scr,
                                op0=AL.mult, op1=AL.mult,
                                accum_out=s2acc[:, o2 * ng + g:o2 * ng + g + 1])
                            nc.vector.tensor_reduce(
                                praw[:, o2, g * 32:(g + 1) * 32].rearrange(
                                    "p (q one) -> p q one", one=1),
                                pm.rearrange("p (q k) -> p q k", k=32),
                                axis=AX.X, op=AL.max)
                stats_to_scale(2, ng, ng)

                # --- BN3 + ReLU on pooled pre-activations; host transposes ---
                for oc in range(4):
                    nc.scalar.activation(pooled[:, oc], praw[:, oc],
                                         AF.Relu, bias=bia[2][:, oc:oc + 1],
                                         scale=scl[2][:, oc:oc + 1])
                nc.sync.dma_start(out=out_d[:, :, :], in_=pooled)

    return nc


def _fix_excess_waits(nc, max_waits=1, nop_waits=1):
    """Walrus allows 1 sync wait on most instructions; hoist excess onto NoOps."""
    for fn in nc.m.functions:
        for blk in fn.blocks:
            new_insts = []
            for ins in blk.instructions:
                si = ins.sync_info
                if si is not None and si.on_wait is not None and len(si.on_wait) > max_waits:
                    waits = list(si.on_wait)
                    extra, keep = waits[:-max_waits], waits[-max_waits:]
                    while extra:
                        chunk, extra = extra[:nop_waits], extra[nop_waits:]
                        nop = mybir.InstNoOp(name=f"{ins.name}-wsplit{len(new_insts)}",
                                             ins=[], outs=[])
                        nop.engine = ins.engine
                        nop.sync_info = mybir.SyncInfo(on_wait=chunk, on_update=[])
                        new_insts.append(nop)
                    ins.sync_info.on_wait = keep
                new_insts.append(ins)
            blk.instructions[:] = new_insts


# ----------------------------------------------------------------------------
# host side
# ----------------------------------------------------------------------------
_CACHE = {}


def _prep_inputs(inputs, n=N, qpc=M * B // NCORES, ncores=NCORES, bm=B * M,
                 b_=B, m_=M):
    fx = np.ascontiguousarray(np.asarray(inputs['ffps_xyz'], np.float32))
    ff = np.ascontiguousarray(np.asarray(inputs['ffps_feature'], np.float32))
    bx = np.ascontiguousarray(np.asarray(inputs['backbone_xyz'], np.float32))
    bf = np.ascontiguousarray(np.asarray(inputs['backbone_features'], np.float32))
    w1 = np.asarray(inputs['w1'], np.float32)
    w2 = np.asarray(inputs['w2'], np.float32)
    w3 = np.asarray(inputs['w3'], np.float32)

    w1aug = np.ascontiguousarray(
        np.concatenate([w1[:, 3:].T, w1[:, :3].T], 0).astype(np.float16))

    w2t = np.ascontiguousarray(w2.T.astype(np.float16))
    w3t = np.ascontiguousarray(w3.T.astype(np.float16))
    sw1tf = np.asarray(inputs['sw1'], np.float32).T
    swh = sw1tf.astype(np.float16)
    swl = (sw1tf - swh.astype(np.float32)).astype(np.float16)
    sw1t2 = np.ascontiguousarray(np.concatenate([swh, swl], 1))
    sw2t = np.ascontiguousarray(np.asarray(inputs['sw2'], np.float32).T)

    bnp = np.zeros((128, 28), np.float32)
    bnp[:, 0] = inputs['sg1']
    bnp[:, 1] = inputs['sb1']
    bnp[0:3, 2] = inputs['sg2']
    bnp[0:3, 3] = inputs['sb2']
    for li, (g, bt) in enumerate(((inputs['g1'], inputs['b1']),
                                  (inputs['g2'], inputs['b2']),
                                  (inputs['g3'], inputs['b3']))):
        g = np.asarray(g, np.float32); bt = np.asarray(bt, np.float32)
        for oc in range(4):
            bnp[:, 4 + 8 * li + oc] = g[oc * 128:(oc + 1) * 128]
            bnp[:, 8 + 8 * li + oc] = bt[oc * 128:(oc + 1) * 128]

    FSH = np.ascontiguousarray(ff.transpose(1, 0, 2).reshape(C, bm))
    XYZT = np.ascontiguousarray(fx.transpose(2, 0, 1).reshape(3, bm))

    cores_per_b = ncores // b_
    in_maps = []
    for c in range(ncores):
        b = c // cores_per_b
        h = c % cores_per_b
        gq0 = b * m_ + h * qpc
        perm = (np.arange(bm) + gq0) % bm
        y = bx[b].T.astype(np.float64)                    # [3, n]
        yh = y.astype(np.float16)
        yl = (y - yh.astype(np.float64)).astype(np.float16)
        ysq = (y ** 2).sum(0)                             # [n] float64
        ysq_h = ysq.astype(np.float16)
        r = ysq - ysq_h.astype(np.float64)
        ysq_m = r.astype(np.float16)
        ysq_l = (r - ysq_m.astype(np.float64)).astype(np.float16)
        yaug2 = np.concatenate(
            [yh, yl, yh, np.ones((3, n), np.float16),
             ysq_h[None], ysq_m[None], ysq_l[None]], 0).astype(np.float16)
        fshp = FSH[:, perm]
        fshp_h = fshp.astype(np.float16)
        fshp_l = (fshp - fshp_h.astype(np.float32)).astype(np.float16)
        in_maps.append({
            'faug': np.ascontiguousarray(
                np.concatenate([bf[b], bx[b].T], 0).astype(np.float16)),
            'yaug': np.ascontiguousarray(yaug2),
            'fshh': np.ascontiguousarray(fshp_h),
            'fshl': np.ascontiguousarray(fshp_l),
            'xyzt': np.ascontiguousarray(XYZT[:, perm]),
            'w1aug': w1aug, 'w2t': w2t, 'w3t': w3t,
            'sw1t2': sw1t2, 'sw2t': sw2t, 'bnp': bnp,
        })
    return in_maps


def kernel(**inputs):
    from concourse.bass_utils import run_bass_kernel_spmd
    if 'nc' not in _CACHE:
        from concourse.library_overlay import lower_extended_insts
        nc = build()
        lower_extended_insts(nc)
        _fix_excess_waits(nc)
        _CACHE['nc'] = nc
    nc = _CACHE['nc']
    in_maps = _prep_inputs(inputs)
    res = run_bass_kernel_spmd(nc, in_maps, list(range(NCORES)))
    qpc = M * B // NCORES
    cores_per_b = NCORES // B
    out = np.empty((B, M, O), np.float32)
    for c in range(NCORES):
        b = c // cores_per_b
        h = c % cores_per_b
        # core output is [128, 4, qpc] fp16: out[q, oc*128 + p] = res[p, oc, q]
        r = res.results[c]["out"].astype(np.float32)          # [128, 4, qpc]
        out[b, h * qpc:(h + 1) * qpc, :] = r.transpose(2, 1, 0).reshape(qpc, O)
    return out

